# revision 62
# baseline (speedup 1.0000x reference)
"""Causal single-head attention (B=4, S=4096, E=1024, D=128) on 8 TRN2 cores.

Sharding: core c = (batch b = c//2, half h = c%2) with ZIG-ZAG causal load
balancing at 256-query granularity. The batch's 16 query groups of 256 are
dealt alternately: core h owns groups j = 2g+h (g = 0..7). Every core's
position-g group needs exactly 4g+4 key blocks of 128, so both cores run
the *same* graph (SPMD).

The key/value pool is host-permuted per core: within each 512-token span u,
the core's own 256 queries come first, the sibling's 256 after. In pool
coordinates the causal structure is then core-independent:
  position g (queries = pool cols [512g, 512g+256)), kb in [0, 4g+4):
    kb < 4g       : fully allowed (no mask)
    kb in {4g,4g+1}: diagonal - compile-time staircase 0/1 mask multiplied
                     into exp(scores) on DVE (bf16)
    kb >= 4g+2    : sibling-half span - allowed iff h=1; gated by a per-core
                     additive bias column (0 / -1e9) fused into ScalarE exp
No collectives are needed.

Compute layout: scores are built transposed ([k, q], key axis on partitions)
into [128, 4, 256] two-bank PSUM quads; one ScalarE exp covers a whole clean
quad (init overhead amortized), masked quads take two half-exps (different
bias). The AV matmul uses exp(scoresT) chunks as the *stationary* operand
and V [k, d] as the moving operand, so (a) the softmax denominator is a
1-row matmul against a ones vector (nearly free on PE) accumulated into a
spare column of the same PSUM bank as AV, and (b) the output lands directly
as [q, d]. Normalization happens on the host (raw AV and denominators are
DMA'd out), which shortens the drain. Softmax skips max-subtraction
(scores stay bounded for randn inputs).

PSUM `start=True` zeroes the whole 2KB bank, so exactly one matmul per
bank-use carries it; later first-writes rely on the pending-zero bytes.

All matmuls run in bf16 (1 cycle/row, f32 PSUM accumulation); fp8 was
measured to push attention-weight noise (~3.7%) straight into the output,
over the 2% budget. x arrives bf16 (halves HBM traffic), V is projected
in [s, d] form (x-chunk stationary, WV moving) - no PE transposes.
Weights arrive pre-arranged in SBUF layout, so no on-chip casts.

Emission is software-pipelined: projection work for later s-groups is
sliced into small "filler" pieces dripped between attention score/AV quads
to absorb the ACT-bound per-quad deficit; each position emits its
diagonal+gated (DVE-hop) quad first.
"""

import sys

if "/opt/trn_rl_repo" not in sys.path:
    sys.path.insert(0, "/opt/trn_rl_repo")

import numpy as np

B, S, E, D = 4, 4096, 1024, 128
H = S // 2  # queries per core
SCALE = 1.0 / 32.0 / 256.0  # 1/sqrt(E); Q,K carry 16x from fp8 packing
NEG = -1.0e9
P = 128  # partitions
QW = 256  # query group width
KB = 128  # key block
ECH = E // P  # e-chunks (8)
NSG = S // 512  # s-groups of 512 over the pool (8)
NQG = H // QW  # q-group positions per core (8)


def _build(nc_args=None):
    import concourse.bass as bass  # noqa: F401
    import concourse.mybir as mybir
    import concourse.tile as tile
    from concourse import bacc

    f32 = mybir.dt.float32
    bf16 = mybir.dt.bfloat16
    f8 = mybir.dt.float8e4

    nc = bacc.Bacc(
        "TRN2",
        target_bir_lowering=False,
        debug=False,
        enable_asserts=False,
        num_devices=8,
    )

    x8_d = nc.dram_tensor("x8", [E, S], f8, kind="ExternalInput").ap()
    r8_d = nc.dram_tensor("r8", [E, S], f8, kind="ExternalInput").ap()
    # packed weights: [proj(k,q,v), slice(W8, rw8), ec, d]; residuals are
    # unscaled fp8 (subnormals carry them), so pass 1 reuses W8
    wp_d = nc.dram_tensor("wp", [P, 3 * 2 * ECH * D], f8, kind="ExternalInput").ap()
    km_d = nc.dram_tensor("km", [P, 2], f32, kind="ExternalInput").ap()
    out_d = nc.dram_tensor("out", [H, D + 1], f32, kind="ExternalOutput").ap()

    with tile.TileContext(nc) as tc:
        from contextlib import ExitStack

        with ExitStack() as ctx:
            consts = ctx.enter_context(tc.tile_pool(name="consts", bufs=1))
            x0_p = ctx.enter_context(tc.tile_pool(name="x0", bufs=1))
            xq_p = ctx.enter_context(tc.tile_pool(name="xq", bufs=4))
            kv_p = ctx.enter_context(tc.tile_pool(name="kv", bufs=1))
            expt_p = ctx.enter_context(tc.tile_pool(name="expt", bufs=8))
            outsb_p = ctx.enter_context(tc.tile_pool(name="outsb", bufs=2))
            ps_sc = ctx.enter_context(tc.tile_pool(name="ps_sc", bufs=2, space="PSUM"))
            ps_proj = ctx.enter_context(
                tc.tile_pool(name="ps_proj", bufs=2, space="PSUM")
            )
            ps_av = ctx.enter_context(tc.tile_pool(name="ps_av", bufs=2, space="PSUM"))

            # ---- weights (pre-arranged [p, proj, pass, ec, d] on host) ----
            wp_sb = consts.tile([P, 3, 2, ECH, D], f8, tag="wp")
            km_sb = consts.tile([P, 2], f32, tag="km")
            PSZ = 2 * ECH * D

            def load_weight(pi, t0=0, t1=2):
                sz = ECH * D
                nc.sync.dma_start(
                    wp_sb[:, pi, t0:t1, :, :].rearrange(
                        "p t ec d -> p (t ec d)"
                    ),
                    wp_d[:, pi * PSZ + t0 * sz : pi * PSZ + t1 * sz],
                )

            ones = consts.tile([P, 1], bf16, tag="ones")
            # combined multiplicative mask for the masked quad (bf16):
            # subtiles 0-1: staircase stair[p, r, f] = (p + r*KB <= f),
            # subtiles 2-3: per-core sibling gate broadcast (0 or 1)
            cmask = consts.tile([P, 4, QW], bf16, tag="cmask")
            nc.gpsimd.memset(cmask[:], 0.0)
            for r in range(2):
                nc.gpsimd.affine_select(
                    out=cmask[:, r, :],
                    in_=cmask[:, r, :],
                    compare_op=mybir.AluOpType.is_ge,
                    fill=1.0,
                    base=r * KB - 1,
                    pattern=[[-1, QW]],
                    channel_multiplier=1,
                )

            # per-s-group projected tiles (s-groups of 512 pool tokens)
            kt_g = [
                kv_p.tile([P, 512], bf16, tag=f"kt{g}", name=f"kt{g}")
                for g in range(NSG)
            ]
            v_g = [
                kv_p.tile([P, 4, D], bf16, tag=f"v{g}", name=f"v{g}")
                for g in range(NSG)
            ]
            qt_g = [
                kv_p.tile([P, QW], bf16, tag=f"qt{g}", name=f"qt{g}")
                for g in range(NQG)
            ]

            x8_re = x8_d.rearrange("(ec p) s -> p ec s", p=P)
            r8_re = r8_d.rearrange("(ec p) s -> p ec s", p=P)
            xtiles = {}  # u -> (x8 pieces, r8 pieces)

            def load_x_quarter(u):
                # stage x8/r8 pool columns [u*1024, (u+1)*1024) as (tile,
                # ec_lo, col_base) pieces; quarter 0 is ec-split (on even
                # boundaries - DoubleRow consumes ec pairs) and ordered by
                # first use: x8 sg0, K pass1-2 W, r8 sg0, Q W, sg1, V W
                both = ([], [])
                if u == 0:
                    def piece(w, re_ap, nm, ec0, ec1, c0, c1):
                        t = x0_p.tile(
                            [P, ec1 - ec0, c1 - c0], f8, tag=f"x0{nm}"
                        )
                        nc.sync.dma_start(t[:], re_ap[:, ec0:ec1, c0:c1])
                        both[w].append((t, ec0, c0))

                    piece(0, x8_re, "ae0", 0, 2, 0, 512)
                    piece(0, x8_re, "aea", 2, ECH, 0, 512)
                    load_weight(0, 1, 2)  # K residual slice
                    piece(1, r8_re, "be0", 0, 2, 0, 512)
                    piece(1, r8_re, "bea", 2, ECH, 0, 512)
                    load_weight(1)  # Q
                    load_weight(2)  # V
                    piece(0, x8_re, "ab", 0, ECH, 512, 1024)
                    piece(1, r8_re, "bb", 0, ECH, 512, 1024)
                else:
                    for half in range(2):
                        for w, (re_ap, nm) in enumerate(
                            ((x8_re, "a"), (r8_re, "b"))
                        ):
                            col = u * 1024 + half * 512
                            t = xq_p.tile(
                                [P, ECH, 512],
                                f8,
                                tag=f"xq{nm}",
                                name=f"xq{nm}{u}_{half}",
                            )
                            nc.sync.dma_start(
                                t[:], re_ap[:, :, col : col + 512]
                            )
                            both[w].append((t, 0, half * 512))
                xtiles[u] = both

            def xsl(u, w, ec_lo, ec_hi, off, width):
                # slice [ec_lo:ec_hi, u*1024+off : +width) of staged x8/r8
                for t, ec_base, col_base in xtiles[u][w]:
                    o = off - col_base
                    e = ec_lo - ec_base
                    if (
                        0 <= o
                        and o + width <= t.shape[2]
                        and 0 <= e
                        and ec_hi - ec_base <= t.shape[1]
                    ):
                        return t[:, e : e + (ec_hi - ec_lo), o : o + width]
                raise AssertionError("bad x slice")

            DR = mybir.MatmulPerfMode.DoubleRow
            # pass t: (x-operand which, weight slice): result accumulates
            # x8@W8 + r8@W8 + x8@rw8 = 16 * x @ W  (compensated fp8;
            # r8/rw8 are unscaled residuals riding e4m3 subnormals)
            PASSES = ((0, 0), (0, 1), (1, 0))  # x8-only passes first

            def project_pieces(sg):
                # K^T [d, s] and V [s, d] for pool tokens [sg*512, (sg+1)*512)
                # and Q^T for position sg (pool cols [512*sg, 512*sg+256)).
                # Returned as small closures so they can be interleaved
                # between attention quads as PE filler work.
                u, off = sg // 2, (sg % 2) * 512
                state = {}

                def kq_pass(pi, key, width, t):
                    xw, wt = PASSES[t]

                    def run():
                        if t == 0:
                            state[key] = ps_proj.tile(
                                [P, 512], f32, tag="proj", name=key
                            )
                        pk = state[key]
                        for j in range(ECH // 2):
                            nc.tensor.matmul(
                                pk[:, 0:width],
                                wp_sb[:, pi, wt, 2 * j : 2 * j + 2, :],
                                xsl(u, xw, 2 * j, 2 * j + 2, off, width),
                                start=(t == 0 and j == 0),
                                stop=(t == 2 and j == ECH // 2 - 1),
                                perf_mode=DR,
                            )
                        if t == 2:
                            pk = state.pop(key)
                            if pi == 0:
                                nc.vector.tensor_copy(kt_g[sg][:], pk[:])
                            else:
                                nc.vector.tensor_copy(
                                    qt_g[sg][:], pk[:, 0:QW]
                                )

                    return run

                def v_t(t):
                    def run():
                        if t == 0:
                            state["pv"] = ps_proj.tile(
                                [P, 512], f32, tag="proj", name="pv"
                            )
                        pv = state["pv"]
                        for ti, (xw, wt) in enumerate(PASSES):
                            for j in range(ECH // 2):
                                nc.tensor.matmul(
                                    pv[:, t * D : (t + 1) * D],
                                    xsl(
                                        u, xw, 2 * j, 2 * j + 2, off + t * P, P
                                    ),
                                    wp_sb[:, 2, wt, 2 * j : 2 * j + 2, :],
                                    start=(t == 0 and ti == 0 and j == 0),
                                    stop=(ti == 2 and j == ECH // 2 - 1),
                                    perf_mode=DR,
                                )
                        if t == 3:
                            pv = state.pop("pv")
                            nc.vector.tensor_copy(
                                v_g[sg][:].rearrange("p t d -> p (t d)"), pv[:]
                            )

                    return run

                return (
                    [kq_pass(0, "pk", 512, t) for t in range(3)]
                    + [kq_pass(1, "pq", QW, t) for t in range(3)]
                    + [v_t(t) for t in range(4)]
                )

            def project_sgroup(sg):
                for piece in project_pieces(sg):
                    piece()

            # ---- attention ----
            att_state = {}
            att_ets = {}

            def att_begin(g):
                # pav [q, d] chunks and den share one PSUM bank: den lives in
                # the spare column D of each chunk
                pavd = ps_av.tile([P, 2, D + 1], f32, tag="avden")
                att_state[g] = pavd

            def att_quad(g, qd):
                # 4 key blocks [4*qd, 4*qd+4) share one 2-bank score tile;
                # clean quads take one [128, 4*QW] exp, the masked quad (qd
                # == g) takes two half-exps (diag bias 0 + stair, gated km)
                psc = ps_sc.tile([P, 4, QW], f32, tag="sc")
                for i in range(4):
                    kb = 4 * qd + i
                    sgk, t = kb // 4, kb % 4
                    nc.tensor.matmul(
                        psc[:, i, :],
                        kt_g[sgk][:, t * KB : (t + 1) * KB],
                        qt_g[g][:],
                        start=True,
                        stop=True,
                    )
                et = expt_p.tile([P, 4, QW], bf16, tag="expt")
                nc.scalar.activation(
                    et[:],
                    psc[:],
                    mybir.ActivationFunctionType.Exp,
                    scale=SCALE,
                )
                if qd == g:
                    nc.vector.tensor_mul(
                        et[:].rearrange("p i q -> p (i q)"),
                        et[:].rearrange("p i q -> p (i q)"),
                        cmask[:].rearrange("p i q -> p (i q)"),
                    )
                att_ets[(g, qd)] = et

            def att_avs(g, qd, first, last):
                pavd = att_state[g]
                et = att_ets.pop((g, qd))
                for i in range(4):
                    kb = 4 * qd + i
                    sgk, t = kb // 4, kb % 4
                    for c in range(2):
                        etc = et[:, i, c * P : (c + 1) * P]
                        nc.tensor.matmul(
                            pavd[:, c, 0:D],
                            etc,
                            v_g[sgk][:, t, :],
                            start=(first and i == 0 and c == 0),
                            stop=(last and i == 3),
                        )
                        nc.tensor.matmul(
                            pavd[:, c, D : D + 1],
                            etc,
                            ones[:],
                            start=False,
                            stop=(last and i == 3),
                        )

            def att_finish(g):
                # raw AV with the denominator in the spare 129th column goes
                # out as-is; normalization happens on the host
                pavd = att_state.pop(g)
                osb = outsb_p.tile([P, 2, D + 1], f32, tag="outsb")
                nc.vector.tensor_copy(osb[:], pavd[:])
                nc.sync.dma_start(
                    out_d[g * QW : (g + 1) * QW, :].rearrange(
                        "(c p) d -> p c d", p=P
                    ),
                    osb[:],
                )

            def att_run(g, fillers=(), lag=4, drip=1, qds=None):
                # quad order: diagonal+gated quad (DVE hop) first, then clean
                # quads; `fillers` are projection pieces for later s-groups,
                # dripped every `drip` quads to keep PE fed while ACT churns
                fillers = list(fillers)
                if qds is None:
                    qds = [g] + list(range(g))
                pend = []
                done = 0
                for n, qd in enumerate(qds):
                    att_quad(g, qd)
                    pend.append(qd)
                    if fillers and n % drip == 0:
                        f = fillers.pop(0)
                        if f is not None:
                            f()
                    if len(pend) > lag:
                        att_avs(g, pend.pop(0), done == 0, done + 1 == len(qds))
                        done += 1
                for qd in pend:
                    att_avs(g, qd, done == 0, done + 1 == len(qds))
                    done += 1
                for f in fillers:
                    f()

            # ---- software-pipelined emission ----
            load_weight(0, 0, 1)  # K pass 0
            load_x_quarter(0)
            nc.sync.dma_start(km_sb[:], km_d[:])
            nc.gpsimd.memset(ones[:], 1.0)
            nc.vector.tensor_scalar_add(
                cmask[:, 2:4, :].rearrange("p i q -> p (i q)"),
                cmask[:, 2:4, :].rearrange("p i q -> p (i q)"),
                km_sb[:, 1:2],
            )
            load_x_quarter(1)
            project_sgroup(0)
            att_begin(0)
            att_run(0, project_pieces(1))
            att_finish(0)
            load_x_quarter(2)
            p6 = project_pieces(6)
            p7 = project_pieces(7)
            for g in range(1, NQG):
                att_begin(g)
                if g + 2 < NSG and g != 5:
                    fillers = project_pieces(g + 1)
                    qds = None
                elif g == 5:
                    # sg6's V is deferred into att6
                    fillers = p6[0:6]
                    qds = None
                elif g == 6:
                    # v6 + q7 drip here; masked quad waits for v6
                    fillers = p6[6:] + p7[3:6]
                    qds = [0, 1, 2, 3, 6, 4, 5]
                else:
                    # sg7's K/V drip inside att7 itself
                    fillers = p7[0:3] + p7[6:]
                    qds = [0, 1, 2, 3, 7, 4, 5, 6]
                att_run(g, fillers, qds=qds)
                att_finish(g)
                if g == 2:
                    load_x_quarter(3)

    nc.compile()
    return nc


_NC = None
LAST_RESULTS = None


def kernel(x, WQ, WK, WV):
    import os

    import ml_dtypes
    from concourse import bass_utils

    global _NC, LAST_RESULTS
    x = np.asarray(x, dtype=np.float32)
    WQ = np.ascontiguousarray(np.asarray(WQ, dtype=np.float32))
    WK = np.ascontiguousarray(np.asarray(WK, dtype=np.float32))
    WV = np.ascontiguousarray(np.asarray(WV, dtype=np.float32))

    if _NC is None:
        _NC = _build()
    nc = _NC

    f8t = ml_dtypes.float8_e4m3

    def sbuf_layout(w):
        # [E, D] -> [P, ECH*D] with e-chunk ec at columns [ec*D, (ec+1)*D)
        return np.ascontiguousarray(
            w.reshape(ECH, P, D).transpose(1, 0, 2).reshape(P, ECH * D)
        )

    def packed_passes(w):
        # compensated fp8: [W8, rw8] of W*16 (rw8 = unscaled residual)
        w16 = sbuf_layout(w * 16.0)
        w8 = w16.astype(f8t)
        rw8 = (w16 - w8.astype(np.float32)).astype(f8t)
        return np.stack([w8, rw8], axis=1)  # [P, 2, ECH*D]

    wp = np.ascontiguousarray(
        np.stack(
            [packed_passes(WK), packed_passes(WQ), packed_passes(WV)], axis=1
        ).reshape(P, 3 * 2 * ECH * D)
    )

    in_maps = []
    for c in range(8):
        b, h = c >> 1, c & 1
        xb = x[b]  # [S, E]
        # pool permutation: per 512-span u, own 256 queries first
        parts = []
        for u in range(8):
            parts.append(xb[512 * u + 256 * h : 512 * u + 256 * h + 256])
            parts.append(
                xb[512 * u + 256 * (1 - h) : 512 * u + 256 * (1 - h) + 256]
            )
        pool_t = np.concatenate(parts, axis=0).T  # [E, S]
        x8 = pool_t.astype(f8t)
        r8 = (pool_t - x8.astype(np.float32)).astype(f8t)
        x8 = np.ascontiguousarray(x8)
        r8 = np.ascontiguousarray(r8)
        km = np.zeros((P, 2), dtype=np.float32)
        km[:, 0] = 0.0 if h == 1 else NEG
        km[:, 1] = float(h)
        in_maps.append({"x8": x8, "r8": r8, "wp": wp, "km": km})

    trace = os.environ.get("KERNEL_TRACE") == "1"
    res = bass_utils.run_bass_kernel_spmd(
        nc, in_maps, core_ids=list(range(8)), trace=trace
    )
    LAST_RESULTS = res

    out = np.empty((B, S, D), dtype=np.float32)
    for c in range(8):
        b, h = c >> 1, c & 1
        raw = res.results[c]["out"]  # [H, D+1]; last column = denominator
        r = raw[:, :D] / raw[:, D:] / 16.0  # V carries 16x from fp8 packing
        for g in range(NQG):
            out[b, 512 * g + 256 * h : 512 * g + 256 * h + 256] = r[
                256 * g : 256 * (g + 1)
            ]
    return out


# revision 65
# speedup vs baseline: 1.0050x; 1.0050x over previous
"""Causal single-head attention (B=4, S=4096, E=1024, D=128) on 8 TRN2 cores.

Sharding: core c = (batch b = c//2, half h = c%2) with ZIG-ZAG causal load
balancing at 256-query granularity. The batch's 16 query groups of 256 are
dealt alternately: core h owns groups j = 2g+h (g = 0..7). Every core's
position-g group needs exactly 4g+4 key blocks of 128, so both cores run
the *same* graph (SPMD).

The key/value pool is host-permuted per core: within each 512-token span u,
the core's own 256 queries come first, the sibling's 256 after. In pool
coordinates the causal structure is then core-independent:
  position g (queries = pool cols [512g, 512g+256)), kb in [0, 4g+4):
    kb < 4g       : fully allowed (no mask)
    kb in {4g,4g+1}: diagonal - compile-time staircase 0/1 mask multiplied
                     into exp(scores) on DVE (bf16)
    kb >= 4g+2    : sibling-half span - allowed iff h=1; gated by a per-core
                     additive bias column (0 / -1e9) fused into ScalarE exp
No collectives are needed.

Compute layout: scores are built transposed ([k, q], key axis on partitions)
into [128, 4, 256] two-bank PSUM quads; one ScalarE exp covers a whole clean
quad (init overhead amortized), masked quads take two half-exps (different
bias). The AV matmul uses exp(scoresT) chunks as the *stationary* operand
and V [k, d] as the moving operand, so (a) the softmax denominator is a
1-row matmul against a ones vector (nearly free on PE) accumulated into a
spare column of the same PSUM bank as AV, and (b) the output lands directly
as [q, d]. Normalization happens on the host (raw AV and denominators are
DMA'd out), which shortens the drain. Softmax skips max-subtraction
(scores stay bounded for randn inputs).

PSUM `start=True` zeroes the whole 2KB bank, so exactly one matmul per
bank-use carries it; later first-writes rely on the pending-zero bytes.

All matmuls run in bf16 (1 cycle/row, f32 PSUM accumulation); fp8 was
measured to push attention-weight noise (~3.7%) straight into the output,
over the 2% budget. x arrives bf16 (halves HBM traffic), V is projected
in [s, d] form (x-chunk stationary, WV moving) - no PE transposes.
Weights arrive pre-arranged in SBUF layout, so no on-chip casts.

Emission is software-pipelined: projection work for later s-groups is
sliced into small "filler" pieces dripped between attention score/AV quads
to absorb the ACT-bound per-quad deficit; each position emits its
diagonal+gated (DVE-hop) quad first.
"""

import sys

if "/opt/trn_rl_repo" not in sys.path:
    sys.path.insert(0, "/opt/trn_rl_repo")

import numpy as np

B, S, E, D = 4, 4096, 1024, 128
H = S // 2  # queries per core
SCALE = 1.0 / 32.0 / 256.0  # 1/sqrt(E); Q,K carry 16x from fp8 packing
NEG = -1.0e9
P = 128  # partitions
QW = 256  # query group width
KB = 128  # key block
ECH = E // P  # e-chunks (8)
NSG = S // 512  # s-groups of 512 over the pool (8)
NQG = H // QW  # q-group positions per core (8)


def _build(nc_args=None):
    import concourse.bass as bass  # noqa: F401
    import concourse.mybir as mybir
    import concourse.tile as tile
    from concourse import bacc

    f32 = mybir.dt.float32
    bf16 = mybir.dt.bfloat16
    f8 = mybir.dt.float8e4

    nc = bacc.Bacc(
        "TRN2",
        target_bir_lowering=False,
        debug=False,
        enable_asserts=False,
        num_devices=8,
    )

    boot_d = nc.dram_tensor("boot", [P, 2 * ECH * D], f8, kind="ExternalInput").ap()
    x8_d = nc.dram_tensor("x8", [E, S], f8, kind="ExternalInput").ap()
    r8_d = nc.dram_tensor("r8", [E, S], f8, kind="ExternalInput").ap()
    # packed weights: [proj(k,q,v), slice(W8, rw8), ec, d]; residuals are
    # unscaled fp8 (subnormals carry them), so pass 1 reuses W8
    wp_d = nc.dram_tensor("wp", [P, 3 * 2 * ECH * D], f8, kind="ExternalInput").ap()
    km_d = nc.dram_tensor("km", [P, 2], f32, kind="ExternalInput").ap()
    out_d = nc.dram_tensor("out", [H, D + 1], f32, kind="ExternalOutput").ap()

    with tile.TileContext(nc) as tc:
        from contextlib import ExitStack

        with ExitStack() as ctx:
            consts = ctx.enter_context(tc.tile_pool(name="consts", bufs=1))
            x0_p = ctx.enter_context(tc.tile_pool(name="x0", bufs=1))
            xq_p = ctx.enter_context(tc.tile_pool(name="xq", bufs=4))
            kv_p = ctx.enter_context(tc.tile_pool(name="kv", bufs=1))
            expt_p = ctx.enter_context(tc.tile_pool(name="expt", bufs=8))
            outsb_p = ctx.enter_context(tc.tile_pool(name="outsb", bufs=2))
            ps_sc = ctx.enter_context(tc.tile_pool(name="ps_sc", bufs=2, space="PSUM"))
            ps_proj = ctx.enter_context(
                tc.tile_pool(name="ps_proj", bufs=2, space="PSUM")
            )
            ps_av = ctx.enter_context(tc.tile_pool(name="ps_av", bufs=2, space="PSUM"))

            # ---- weights (pre-arranged [p, proj, pass, ec, d] on host) ----
            # boot: K's W8 and the first x piece ride ONE DMA so the very
            # first matmul waits a single transfer chain
            boot_sb = consts.tile([P, 2 * ECH * D], f8, tag="boot")
            kw8 = boot_sb[:, 0 : ECH * D].rearrange("p (ec d) -> p ec d", d=D)
            bootx = boot_sb[:, ECH * D :].rearrange("p (ec s) -> p ec s", s=512)
            wp_sb = consts.tile([P, 3, 2, ECH, D], f8, tag="wp")
            km_sb = consts.tile([P, 2], f32, tag="km")
            PSZ = 2 * ECH * D

            def load_weight(pi, t0=0, t1=2):
                sz = ECH * D
                nc.sync.dma_start(
                    wp_sb[:, pi, t0:t1, :, :].rearrange(
                        "p t ec d -> p (t ec d)"
                    ),
                    wp_d[:, pi * PSZ + t0 * sz : pi * PSZ + t1 * sz],
                )

            ones = consts.tile([P, 1], bf16, tag="ones")
            # combined multiplicative mask for the masked quad (bf16):
            # subtiles 0-1: staircase stair[p, r, f] = (p + r*KB <= f),
            # subtiles 2-3: per-core sibling gate broadcast (0 or 1)
            cmask = consts.tile([P, 4, QW], bf16, tag="cmask")
            nc.gpsimd.memset(cmask[:], 0.0)
            for r in range(2):
                nc.gpsimd.affine_select(
                    out=cmask[:, r, :],
                    in_=cmask[:, r, :],
                    compare_op=mybir.AluOpType.is_ge,
                    fill=1.0,
                    base=r * KB - 1,
                    pattern=[[-1, QW]],
                    channel_multiplier=1,
                )

            # per-s-group projected tiles (s-groups of 512 pool tokens)
            kt_g = [
                kv_p.tile([P, 512], bf16, tag=f"kt{g}", name=f"kt{g}")
                for g in range(NSG)
            ]
            v_g = [
                kv_p.tile([P, 4, D], bf16, tag=f"v{g}", name=f"v{g}")
                for g in range(NSG)
            ]
            qt_g = [
                kv_p.tile([P, QW], bf16, tag=f"qt{g}", name=f"qt{g}")
                for g in range(NQG)
            ]

            x8_re = x8_d.rearrange("(ec p) s -> p ec s", p=P)
            r8_re = r8_d.rearrange("(ec p) s -> p ec s", p=P)
            xtiles = {}  # u -> (x8 pieces, r8 pieces)

            def load_x_quarter(u):
                # stage x8/r8 pool columns [u*1024, (u+1)*1024) as (tile,
                # ec_lo, col_base) pieces; quarter 0 is ec-split (on even
                # boundaries - DoubleRow consumes ec pairs) and ordered by
                # first use: x8 sg0, K pass1-2 W, r8 sg0, Q W, sg1, V W
                both = ([], [])
                if u == 0:
                    def piece(w, re_ap, nm, ec0, ec1, c0, c1):
                        t = x0_p.tile(
                            [P, ec1 - ec0, c1 - c0], f8, tag=f"x0{nm}"
                        )
                        nc.sync.dma_start(t[:], re_ap[:, ec0:ec1, c0:c1])
                        both[w].append((t, ec0, c0))

                    both[0].append((bootx, 0, 0))  # x8 ec0-1 via boot DMA
                    piece(0, x8_re, "aea", 2, ECH, 0, 512)
                    load_weight(0, 1, 2)  # K residual slice
                    piece(1, r8_re, "be0", 0, 2, 0, 512)
                    piece(1, r8_re, "bea", 2, ECH, 0, 512)
                    load_weight(1)  # Q
                    load_weight(2)  # V
                    piece(0, x8_re, "ab", 0, ECH, 512, 1024)
                    piece(1, r8_re, "bb", 0, ECH, 512, 1024)
                else:
                    for half in range(2):
                        for w, (re_ap, nm) in enumerate(
                            ((x8_re, "a"), (r8_re, "b"))
                        ):
                            col = u * 1024 + half * 512
                            t = xq_p.tile(
                                [P, ECH, 512],
                                f8,
                                tag=f"xq{nm}",
                                name=f"xq{nm}{u}_{half}",
                            )
                            nc.sync.dma_start(
                                t[:], re_ap[:, :, col : col + 512]
                            )
                            both[w].append((t, 0, half * 512))
                xtiles[u] = both

            def xsl(u, w, ec_lo, ec_hi, off, width):
                # slice [ec_lo:ec_hi, u*1024+off : +width) of staged x8/r8
                for t, ec_base, col_base in xtiles[u][w]:
                    o = off - col_base
                    e = ec_lo - ec_base
                    if (
                        0 <= o
                        and o + width <= t.shape[2]
                        and 0 <= e
                        and ec_hi - ec_base <= t.shape[1]
                    ):
                        return t[:, e : e + (ec_hi - ec_lo), o : o + width]
                raise AssertionError("bad x slice")

            def wsl(pi, wt):
                # K's W8 slice lives in the boot tile
                if pi == 0 and wt == 0:
                    return kw8
                return wp_sb[:, pi, wt, :, :]

            DR = mybir.MatmulPerfMode.DoubleRow
            # pass t: (x-operand which, weight slice): result accumulates
            # x8@W8 + r8@W8 + x8@rw8 = 16 * x @ W  (compensated fp8;
            # r8/rw8 are unscaled residuals riding e4m3 subnormals)
            PASSES = ((0, 0), (0, 1), (1, 0))  # x8-only passes first

            def project_pieces(sg):
                # K^T [d, s] and V [s, d] for pool tokens [sg*512, (sg+1)*512)
                # and Q^T for position sg (pool cols [512*sg, 512*sg+256)).
                # Returned as small closures so they can be interleaved
                # between attention quads as PE filler work.
                u, off = sg // 2, (sg % 2) * 512
                state = {}

                def kq_pass(pi, key, width, t):
                    xw, wt = PASSES[t]

                    def run():
                        if t == 0:
                            state[key] = ps_proj.tile(
                                [P, 512], f32, tag="proj", name=key
                            )
                        pk = state[key]
                        for j in range(ECH // 2):
                            nc.tensor.matmul(
                                pk[:, 0:width],
                                wsl(pi, wt)[:, 2 * j : 2 * j + 2, :],
                                xsl(u, xw, 2 * j, 2 * j + 2, off, width),
                                start=(t == 0 and j == 0),
                                stop=(t == 2 and j == ECH // 2 - 1),
                                perf_mode=DR,
                            )
                        if t == 2:
                            pk = state.pop(key)
                            if pi == 0:
                                nc.vector.tensor_copy(kt_g[sg][:], pk[:])
                            else:
                                nc.vector.tensor_copy(
                                    qt_g[sg][:], pk[:, 0:QW]
                                )

                    return run

                def v_t(t):
                    def run():
                        if t == 0:
                            state["pv"] = ps_proj.tile(
                                [P, 512], f32, tag="proj", name="pv"
                            )
                        pv = state["pv"]
                        for ti, (xw, wt) in enumerate(PASSES):
                            for j in range(ECH // 2):
                                nc.tensor.matmul(
                                    pv[:, t * D : (t + 1) * D],
                                    xsl(
                                        u, xw, 2 * j, 2 * j + 2, off + t * P, P
                                    ),
                                    wsl(2, wt)[:, 2 * j : 2 * j + 2, :],
                                    start=(t == 0 and ti == 0 and j == 0),
                                    stop=(ti == 2 and j == ECH // 2 - 1),
                                    perf_mode=DR,
                                )
                        if t == 3:
                            pv = state.pop("pv")
                            nc.vector.tensor_copy(
                                v_g[sg][:].rearrange("p t d -> p (t d)"), pv[:]
                            )

                    return run

                return (
                    [kq_pass(0, "pk", 512, t) for t in range(3)]
                    + [kq_pass(1, "pq", QW, t) for t in range(3)]
                    + [v_t(t) for t in range(4)]
                )

            def project_sgroup(sg):
                for piece in project_pieces(sg):
                    piece()

            # ---- attention ----
            att_state = {}
            att_ets = {}

            def att_begin(g):
                # pav [q, d] chunks and den share one PSUM bank: den lives in
                # the spare column D of each chunk
                pavd = ps_av.tile([P, 2, D + 1], f32, tag="avden")
                att_state[g] = pavd

            def att_quad(g, qd):
                # 4 key blocks [4*qd, 4*qd+4) share one 2-bank score tile;
                # clean quads take one [128, 4*QW] exp, the masked quad (qd
                # == g) takes two half-exps (diag bias 0 + stair, gated km)
                psc = ps_sc.tile([P, 4, QW], f32, tag="sc")
                for i in range(4):
                    kb = 4 * qd + i
                    sgk, t = kb // 4, kb % 4
                    nc.tensor.matmul(
                        psc[:, i, :],
                        kt_g[sgk][:, t * KB : (t + 1) * KB],
                        qt_g[g][:],
                        start=True,
                        stop=True,
                    )
                et = expt_p.tile([P, 4, QW], bf16, tag="expt")
                nc.scalar.activation(
                    et[:],
                    psc[:],
                    mybir.ActivationFunctionType.Exp,
                    scale=SCALE,
                )
                if qd == g:
                    nc.vector.tensor_mul(
                        et[:].rearrange("p i q -> p (i q)"),
                        et[:].rearrange("p i q -> p (i q)"),
                        cmask[:].rearrange("p i q -> p (i q)"),
                    )
                att_ets[(g, qd)] = et

            def att_avs(g, qd, first, last):
                pavd = att_state[g]
                et = att_ets.pop((g, qd))
                for i in range(4):
                    kb = 4 * qd + i
                    sgk, t = kb // 4, kb % 4
                    for c in range(2):
                        etc = et[:, i, c * P : (c + 1) * P]
                        nc.tensor.matmul(
                            pavd[:, c, 0:D],
                            etc,
                            v_g[sgk][:, t, :],
                            start=(first and i == 0 and c == 0),
                            stop=(last and i == 3),
                        )
                        nc.tensor.matmul(
                            pavd[:, c, D : D + 1],
                            etc,
                            ones[:],
                            start=False,
                            stop=(last and i == 3),
                        )

            def att_finish(g):
                # raw AV with the denominator in the spare 129th column goes
                # out as-is; normalization happens on the host
                pavd = att_state.pop(g)
                osb = outsb_p.tile([P, 2, D + 1], f32, tag="outsb")
                nc.vector.tensor_copy(osb[:], pavd[:])
                nc.sync.dma_start(
                    out_d[g * QW : (g + 1) * QW, :].rearrange(
                        "(c p) d -> p c d", p=P
                    ),
                    osb[:],
                )

            def att_run(g, fillers=(), lag=4, drip=1, qds=None):
                # quad order: diagonal+gated quad (DVE hop) first, then clean
                # quads; `fillers` are projection pieces for later s-groups,
                # dripped every `drip` quads to keep PE fed while ACT churns
                fillers = list(fillers)
                if qds is None:
                    qds = [g] + list(range(g))
                pend = []
                done = 0
                for n, qd in enumerate(qds):
                    att_quad(g, qd)
                    pend.append(qd)
                    if fillers and n % drip == 0:
                        f = fillers.pop(0)
                        if f is not None:
                            f()
                    if len(pend) > lag:
                        att_avs(g, pend.pop(0), done == 0, done + 1 == len(qds))
                        done += 1
                for qd in pend:
                    att_avs(g, qd, done == 0, done + 1 == len(qds))
                    done += 1
                for f in fillers:
                    f()

            # ---- software-pipelined emission ----
            nc.sync.dma_start(boot_sb[:], boot_d[:])
            load_x_quarter(0)
            nc.sync.dma_start(km_sb[:], km_d[:])
            nc.gpsimd.memset(ones[:], 1.0)
            nc.vector.tensor_scalar_add(
                cmask[:, 2:4, :].rearrange("p i q -> p (i q)"),
                cmask[:, 2:4, :].rearrange("p i q -> p (i q)"),
                km_sb[:, 1:2],
            )
            load_x_quarter(1)
            project_sgroup(0)
            att_begin(0)
            att_run(0, project_pieces(1))
            att_finish(0)
            load_x_quarter(2)
            p6 = project_pieces(6)
            p7 = project_pieces(7)
            for g in range(1, NQG):
                att_begin(g)
                if g + 2 < NSG and g != 5:
                    fillers = project_pieces(g + 1)
                    qds = None
                elif g == 5:
                    # sg6's V is deferred into att6
                    fillers = p6[0:6]
                    qds = None
                elif g == 6:
                    # v6 + q7 drip here; masked quad waits for v6
                    fillers = p6[6:] + p7[3:6]
                    qds = [0, 1, 2, 3, 6, 4, 5]
                else:
                    # sg7's K/V drip inside att7 itself
                    fillers = p7[0:3] + p7[6:]
                    qds = [0, 1, 2, 3, 7, 4, 5, 6]
                att_run(g, fillers, qds=qds)
                att_finish(g)
                if g == 2:
                    load_x_quarter(3)

    nc.compile()
    return nc


_NC = None
LAST_RESULTS = None


def kernel(x, WQ, WK, WV):
    import os

    import ml_dtypes
    from concourse import bass_utils

    global _NC, LAST_RESULTS
    x = np.asarray(x, dtype=np.float32)
    WQ = np.ascontiguousarray(np.asarray(WQ, dtype=np.float32))
    WK = np.ascontiguousarray(np.asarray(WK, dtype=np.float32))
    WV = np.ascontiguousarray(np.asarray(WV, dtype=np.float32))

    if _NC is None:
        _NC = _build()
    nc = _NC

    f8t = ml_dtypes.float8_e4m3

    def sbuf_layout(w):
        # [E, D] -> [P, ECH*D] with e-chunk ec at columns [ec*D, (ec+1)*D)
        return np.ascontiguousarray(
            w.reshape(ECH, P, D).transpose(1, 0, 2).reshape(P, ECH * D)
        )

    def packed_passes(w):
        # compensated fp8: [W8, rw8] of W*16 (rw8 = unscaled residual)
        w16 = sbuf_layout(w * 16.0)
        w8 = w16.astype(f8t)
        rw8 = (w16 - w8.astype(np.float32)).astype(f8t)
        return np.stack([w8, rw8], axis=1)  # [P, 2, ECH*D]

    wp = np.ascontiguousarray(
        np.stack(
            [packed_passes(WK), packed_passes(WQ), packed_passes(WV)], axis=1
        ).reshape(P, 3 * 2 * ECH * D)
    )

    in_maps = []
    for c in range(8):
        b, h = c >> 1, c & 1
        xb = x[b]  # [S, E]
        # pool permutation: per 512-span u, own 256 queries first
        parts = []
        for u in range(8):
            parts.append(xb[512 * u + 256 * h : 512 * u + 256 * h + 256])
            parts.append(
                xb[512 * u + 256 * (1 - h) : 512 * u + 256 * (1 - h) + 256]
            )
        pool_t = np.concatenate(parts, axis=0).T  # [E, S]
        x8 = pool_t.astype(f8t)
        r8 = (pool_t - x8.astype(np.float32)).astype(f8t)
        x8 = np.ascontiguousarray(x8)
        r8 = np.ascontiguousarray(r8)
        # boot: [K W8 | x8 ec0-1, cols 0:512] per partition
        xpart = (
            x8[: 2 * P, 0:512]
            .reshape(2, P, 512)
            .transpose(1, 0, 2)
            .reshape(P, 1024)
        )
        boot = np.ascontiguousarray(
            np.concatenate([wp[:, 0:1024], xpart], axis=1)
        )
        km = np.zeros((P, 2), dtype=np.float32)
        km[:, 0] = 0.0 if h == 1 else NEG
        km[:, 1] = float(h)
        in_maps.append(
            {"boot": boot, "x8": x8, "r8": r8, "wp": wp, "km": km}
        )

    trace = os.environ.get("KERNEL_TRACE") == "1"
    res = bass_utils.run_bass_kernel_spmd(
        nc, in_maps, core_ids=list(range(8)), trace=trace
    )
    LAST_RESULTS = res

    out = np.empty((B, S, D), dtype=np.float32)
    for c in range(8):
        b, h = c >> 1, c & 1
        raw = res.results[c]["out"]  # [H, D+1]; last column = denominator
        r = raw[:, :D] / raw[:, D:] / 16.0  # V carries 16x from fp8 packing
        for g in range(NQG):
            out[b, 512 * g + 256 * h : 512 * g + 256 * h + 256] = r[
                256 * g : 256 * (g + 1)
            ]
    return out


# revision 66
# speedup vs baseline: 1.0055x; 1.0005x over previous
"""Causal single-head attention (B=4, S=4096, E=1024, D=128) on 8 TRN2 cores.

Sharding: core c = (batch b = c//2, half h = c%2) with ZIG-ZAG causal load
balancing at 256-query granularity. The batch's 16 query groups of 256 are
dealt alternately: core h owns groups j = 2g+h (g = 0..7). Every core's
position-g group needs exactly 4g+4 key blocks of 128, so both cores run
the *same* graph (SPMD).

The key/value pool is host-permuted per core: within each 512-token span u,
the core's own 256 queries come first, the sibling's 256 after. In pool
coordinates the causal structure is then core-independent:
  position g (queries = pool cols [512g, 512g+256)), kb in [0, 4g+4):
    kb < 4g       : fully allowed (no mask)
    kb in {4g,4g+1}: diagonal - compile-time staircase 0/1 mask multiplied
                     into exp(scores) on DVE (bf16)
    kb >= 4g+2    : sibling-half span - allowed iff h=1; gated by a per-core
                     additive bias column (0 / -1e9) fused into ScalarE exp
No collectives are needed.

Compute layout: scores are built transposed ([k, q], key axis on partitions)
into [128, 4, 256] two-bank PSUM quads; one ScalarE exp covers a whole clean
quad (init overhead amortized), masked quads take two half-exps (different
bias). The AV matmul uses exp(scoresT) chunks as the *stationary* operand
and V [k, d] as the moving operand, so (a) the softmax denominator is a
1-row matmul against a ones vector (nearly free on PE) accumulated into a
spare column of the same PSUM bank as AV, and (b) the output lands directly
as [q, d]. Normalization happens on the host (raw AV and denominators are
DMA'd out), which shortens the drain. Softmax skips max-subtraction
(scores stay bounded for randn inputs).

PSUM `start=True` zeroes the whole 2KB bank, so exactly one matmul per
bank-use carries it; later first-writes rely on the pending-zero bytes.

All matmuls run in bf16 (1 cycle/row, f32 PSUM accumulation); fp8 was
measured to push attention-weight noise (~3.7%) straight into the output,
over the 2% budget. x arrives bf16 (halves HBM traffic), V is projected
in [s, d] form (x-chunk stationary, WV moving) - no PE transposes.
Weights arrive pre-arranged in SBUF layout, so no on-chip casts.

Emission is software-pipelined: projection work for later s-groups is
sliced into small "filler" pieces dripped between attention score/AV quads
to absorb the ACT-bound per-quad deficit; each position emits its
diagonal+gated (DVE-hop) quad first.
"""

import sys

if "/opt/trn_rl_repo" not in sys.path:
    sys.path.insert(0, "/opt/trn_rl_repo")

import numpy as np

B, S, E, D = 4, 4096, 1024, 128
H = S // 2  # queries per core
SCALE = 1.0 / 32.0 / 256.0  # 1/sqrt(E); Q,K carry 16x from fp8 packing
NEG = -1.0e9
P = 128  # partitions
QW = 256  # query group width
KB = 128  # key block
ECH = E // P  # e-chunks (8)
NSG = S // 512  # s-groups of 512 over the pool (8)
NQG = H // QW  # q-group positions per core (8)


def _build(nc_args=None):
    import concourse.bass as bass  # noqa: F401
    import concourse.mybir as mybir
    import concourse.tile as tile
    from concourse import bacc

    f32 = mybir.dt.float32
    bf16 = mybir.dt.bfloat16
    f8 = mybir.dt.float8e4

    nc = bacc.Bacc(
        "TRN2",
        target_bir_lowering=False,
        debug=False,
        enable_asserts=False,
        num_devices=8,
    )

    boot_d = nc.dram_tensor("boot", [P, 3 * ECH * D], f8, kind="ExternalInput").ap()
    x8_d = nc.dram_tensor("x8", [E, S], f8, kind="ExternalInput").ap()
    r8_d = nc.dram_tensor("r8", [E, S], f8, kind="ExternalInput").ap()
    # packed weights: [proj(k,q,v), slice(W8, rw8), ec, d]; residuals are
    # unscaled fp8 (subnormals carry them), so pass 1 reuses W8
    wp_d = nc.dram_tensor("wp", [P, 3 * 2 * ECH * D], f8, kind="ExternalInput").ap()
    km_d = nc.dram_tensor("km", [P, 2], f32, kind="ExternalInput").ap()
    out_d = nc.dram_tensor("out", [H, D + 1], f32, kind="ExternalOutput").ap()

    with tile.TileContext(nc) as tc:
        from contextlib import ExitStack

        with ExitStack() as ctx:
            consts = ctx.enter_context(tc.tile_pool(name="consts", bufs=1))
            x0_p = ctx.enter_context(tc.tile_pool(name="x0", bufs=1))
            xq_p = ctx.enter_context(tc.tile_pool(name="xq", bufs=4))
            kv_p = ctx.enter_context(tc.tile_pool(name="kv", bufs=1))
            expt_p = ctx.enter_context(tc.tile_pool(name="expt", bufs=8))
            outsb_p = ctx.enter_context(tc.tile_pool(name="outsb", bufs=2))
            ps_sc = ctx.enter_context(tc.tile_pool(name="ps_sc", bufs=2, space="PSUM"))
            ps_proj = ctx.enter_context(
                tc.tile_pool(name="ps_proj", bufs=2, space="PSUM")
            )
            ps_av = ctx.enter_context(tc.tile_pool(name="ps_av", bufs=2, space="PSUM"))

            # ---- weights (pre-arranged [p, proj, pass, ec, d] on host) ----
            # boot: K's W8 and the first x piece ride ONE DMA so the very
            # first matmul waits a single transfer chain
            boot_sb = consts.tile([P, 3 * ECH * D], f8, tag="boot")
            kw8 = boot_sb[:, 0 : ECH * D].rearrange("p (ec d) -> p ec d", d=D)
            bootx = boot_sb[:, ECH * D :].rearrange("p (ec s) -> p ec s", s=512)
            wp_sb = consts.tile([P, 3, 2, ECH, D], f8, tag="wp")
            km_sb = consts.tile([P, 2], f32, tag="km")
            PSZ = 2 * ECH * D

            def load_weight(pi, t0=0, t1=2):
                sz = ECH * D
                nc.sync.dma_start(
                    wp_sb[:, pi, t0:t1, :, :].rearrange(
                        "p t ec d -> p (t ec d)"
                    ),
                    wp_d[:, pi * PSZ + t0 * sz : pi * PSZ + t1 * sz],
                )

            ones = consts.tile([P, 1], bf16, tag="ones")
            # combined multiplicative mask for the masked quad (bf16):
            # subtiles 0-1: staircase stair[p, r, f] = (p + r*KB <= f),
            # subtiles 2-3: per-core sibling gate broadcast (0 or 1)
            cmask = consts.tile([P, 4, QW], bf16, tag="cmask")
            nc.gpsimd.memset(cmask[:], 0.0)
            for r in range(2):
                nc.gpsimd.affine_select(
                    out=cmask[:, r, :],
                    in_=cmask[:, r, :],
                    compare_op=mybir.AluOpType.is_ge,
                    fill=1.0,
                    base=r * KB - 1,
                    pattern=[[-1, QW]],
                    channel_multiplier=1,
                )

            # per-s-group projected tiles (s-groups of 512 pool tokens)
            kt_g = [
                kv_p.tile([P, 512], bf16, tag=f"kt{g}", name=f"kt{g}")
                for g in range(NSG)
            ]
            v_g = [
                kv_p.tile([P, 4, D], bf16, tag=f"v{g}", name=f"v{g}")
                for g in range(NSG)
            ]
            qt_g = [
                kv_p.tile([P, QW], bf16, tag=f"qt{g}", name=f"qt{g}")
                for g in range(NQG)
            ]

            x8_re = x8_d.rearrange("(ec p) s -> p ec s", p=P)
            r8_re = r8_d.rearrange("(ec p) s -> p ec s", p=P)
            xtiles = {}  # u -> (x8 pieces, r8 pieces)

            def load_x_quarter(u):
                # stage x8/r8 pool columns [u*1024, (u+1)*1024) as (tile,
                # ec_lo, col_base) pieces; quarter 0 is ec-split (on even
                # boundaries - DoubleRow consumes ec pairs) and ordered by
                # first use: x8 sg0, K pass1-2 W, r8 sg0, Q W, sg1, V W
                both = ([], [])
                if u == 0:
                    def piece(w, re_ap, nm, ec0, ec1, c0, c1):
                        t = x0_p.tile(
                            [P, ec1 - ec0, c1 - c0], f8, tag=f"x0{nm}"
                        )
                        nc.sync.dma_start(t[:], re_ap[:, ec0:ec1, c0:c1])
                        both[w].append((t, ec0, c0))

                    both[0].append((bootx, 0, 0))  # x8 ec0-3 via boot DMA
                    piece(0, x8_re, "aea", 4, ECH, 0, 512)
                    load_weight(0, 1, 2)  # K residual slice
                    piece(1, r8_re, "be0", 0, 2, 0, 512)
                    piece(1, r8_re, "bea", 2, ECH, 0, 512)
                    load_weight(1)  # Q
                    load_weight(2)  # V
                    piece(0, x8_re, "ab", 0, ECH, 512, 1024)
                    piece(1, r8_re, "bb", 0, ECH, 512, 1024)
                else:
                    for half in range(2):
                        for w, (re_ap, nm) in enumerate(
                            ((x8_re, "a"), (r8_re, "b"))
                        ):
                            col = u * 1024 + half * 512
                            t = xq_p.tile(
                                [P, ECH, 512],
                                f8,
                                tag=f"xq{nm}",
                                name=f"xq{nm}{u}_{half}",
                            )
                            nc.sync.dma_start(
                                t[:], re_ap[:, :, col : col + 512]
                            )
                            both[w].append((t, 0, half * 512))
                xtiles[u] = both

            def xsl(u, w, ec_lo, ec_hi, off, width):
                # slice [ec_lo:ec_hi, u*1024+off : +width) of staged x8/r8
                for t, ec_base, col_base in xtiles[u][w]:
                    o = off - col_base
                    e = ec_lo - ec_base
                    if (
                        0 <= o
                        and o + width <= t.shape[2]
                        and 0 <= e
                        and ec_hi - ec_base <= t.shape[1]
                    ):
                        return t[:, e : e + (ec_hi - ec_lo), o : o + width]
                raise AssertionError("bad x slice")

            def wsl(pi, wt):
                # K's W8 slice lives in the boot tile
                if pi == 0 and wt == 0:
                    return kw8
                return wp_sb[:, pi, wt, :, :]

            DR = mybir.MatmulPerfMode.DoubleRow
            # pass t: (x-operand which, weight slice): result accumulates
            # x8@W8 + r8@W8 + x8@rw8 = 16 * x @ W  (compensated fp8;
            # r8/rw8 are unscaled residuals riding e4m3 subnormals)
            PASSES = ((0, 0), (0, 1), (1, 0))  # x8-only passes first

            def project_pieces(sg):
                # K^T [d, s] and V [s, d] for pool tokens [sg*512, (sg+1)*512)
                # and Q^T for position sg (pool cols [512*sg, 512*sg+256)).
                # Returned as small closures so they can be interleaved
                # between attention quads as PE filler work.
                u, off = sg // 2, (sg % 2) * 512
                state = {}

                def kq_pass(pi, key, width, t):
                    xw, wt = PASSES[t]

                    def run():
                        if t == 0:
                            state[key] = ps_proj.tile(
                                [P, 512], f32, tag="proj", name=key
                            )
                        pk = state[key]
                        for j in range(ECH // 2):
                            nc.tensor.matmul(
                                pk[:, 0:width],
                                wsl(pi, wt)[:, 2 * j : 2 * j + 2, :],
                                xsl(u, xw, 2 * j, 2 * j + 2, off, width),
                                start=(t == 0 and j == 0),
                                stop=(t == 2 and j == ECH // 2 - 1),
                                perf_mode=DR,
                            )
                        if t == 2:
                            pk = state.pop(key)
                            if pi == 0:
                                nc.vector.tensor_copy(kt_g[sg][:], pk[:])
                            else:
                                nc.vector.tensor_copy(
                                    qt_g[sg][:], pk[:, 0:QW]
                                )

                    return run

                def v_t(t):
                    def run():
                        if t == 0:
                            state["pv"] = ps_proj.tile(
                                [P, 512], f32, tag="proj", name="pv"
                            )
                        pv = state["pv"]
                        for ti, (xw, wt) in enumerate(PASSES):
                            for j in range(ECH // 2):
                                nc.tensor.matmul(
                                    pv[:, t * D : (t + 1) * D],
                                    xsl(
                                        u, xw, 2 * j, 2 * j + 2, off + t * P, P
                                    ),
                                    wsl(2, wt)[:, 2 * j : 2 * j + 2, :],
                                    start=(t == 0 and ti == 0 and j == 0),
                                    stop=(ti == 2 and j == ECH // 2 - 1),
                                    perf_mode=DR,
                                )
                        if t == 3:
                            pv = state.pop("pv")
                            nc.vector.tensor_copy(
                                v_g[sg][:].rearrange("p t d -> p (t d)"), pv[:]
                            )

                    return run

                return (
                    [kq_pass(0, "pk", 512, t) for t in range(3)]
                    + [kq_pass(1, "pq", QW, t) for t in range(3)]
                    + [v_t(t) for t in range(4)]
                )

            def project_sgroup(sg):
                for piece in project_pieces(sg):
                    piece()

            # ---- attention ----
            att_state = {}
            att_ets = {}

            def att_begin(g):
                # pav [q, d] chunks and den share one PSUM bank: den lives in
                # the spare column D of each chunk
                pavd = ps_av.tile([P, 2, D + 1], f32, tag="avden")
                att_state[g] = pavd

            def att_quad(g, qd):
                # 4 key blocks [4*qd, 4*qd+4) share one 2-bank score tile;
                # clean quads take one [128, 4*QW] exp, the masked quad (qd
                # == g) takes two half-exps (diag bias 0 + stair, gated km)
                psc = ps_sc.tile([P, 4, QW], f32, tag="sc")
                for i in range(4):
                    kb = 4 * qd + i
                    sgk, t = kb // 4, kb % 4
                    nc.tensor.matmul(
                        psc[:, i, :],
                        kt_g[sgk][:, t * KB : (t + 1) * KB],
                        qt_g[g][:],
                        start=True,
                        stop=True,
                    )
                et = expt_p.tile([P, 4, QW], bf16, tag="expt")
                nc.scalar.activation(
                    et[:],
                    psc[:],
                    mybir.ActivationFunctionType.Exp,
                    scale=SCALE,
                )
                if qd == g:
                    nc.vector.tensor_mul(
                        et[:].rearrange("p i q -> p (i q)"),
                        et[:].rearrange("p i q -> p (i q)"),
                        cmask[:].rearrange("p i q -> p (i q)"),
                    )
                att_ets[(g, qd)] = et

            def att_avs(g, qd, first, last):
                pavd = att_state[g]
                et = att_ets.pop((g, qd))
                for i in range(4):
                    kb = 4 * qd + i
                    sgk, t = kb // 4, kb % 4
                    for c in range(2):
                        etc = et[:, i, c * P : (c + 1) * P]
                        nc.tensor.matmul(
                            pavd[:, c, 0:D],
                            etc,
                            v_g[sgk][:, t, :],
                            start=(first and i == 0 and c == 0),
                            stop=(last and i == 3),
                        )
                        nc.tensor.matmul(
                            pavd[:, c, D : D + 1],
                            etc,
                            ones[:],
                            start=False,
                            stop=(last and i == 3),
                        )

            def att_finish(g):
                # raw AV with the denominator in the spare 129th column goes
                # out as-is; normalization happens on the host
                pavd = att_state.pop(g)
                osb = outsb_p.tile([P, 2, D + 1], f32, tag="outsb")
                nc.vector.tensor_copy(osb[:], pavd[:])
                nc.sync.dma_start(
                    out_d[g * QW : (g + 1) * QW, :].rearrange(
                        "(c p) d -> p c d", p=P
                    ),
                    osb[:],
                )

            def att_run(g, fillers=(), lag=4, drip=1, qds=None):
                # quad order: diagonal+gated quad (DVE hop) first, then clean
                # quads; `fillers` are projection pieces for later s-groups,
                # dripped every `drip` quads to keep PE fed while ACT churns
                fillers = list(fillers)
                if qds is None:
                    qds = [g] + list(range(g))
                pend = []
                done = 0
                for n, qd in enumerate(qds):
                    att_quad(g, qd)
                    pend.append(qd)
                    if fillers and n % drip == 0:
                        f = fillers.pop(0)
                        if f is not None:
                            f()
                    if len(pend) > lag:
                        att_avs(g, pend.pop(0), done == 0, done + 1 == len(qds))
                        done += 1
                for qd in pend:
                    att_avs(g, qd, done == 0, done + 1 == len(qds))
                    done += 1
                for f in fillers:
                    f()

            # ---- software-pipelined emission ----
            nc.sync.dma_start(boot_sb[:], boot_d[:])
            load_x_quarter(0)
            nc.sync.dma_start(km_sb[:], km_d[:])
            nc.gpsimd.memset(ones[:], 1.0)
            nc.vector.tensor_scalar_add(
                cmask[:, 2:4, :].rearrange("p i q -> p (i q)"),
                cmask[:, 2:4, :].rearrange("p i q -> p (i q)"),
                km_sb[:, 1:2],
            )
            load_x_quarter(1)
            project_sgroup(0)
            att_begin(0)
            att_run(0, project_pieces(1))
            att_finish(0)
            load_x_quarter(2)
            p6 = project_pieces(6)
            p7 = project_pieces(7)
            for g in range(1, NQG):
                att_begin(g)
                if g + 2 < NSG and g != 5:
                    fillers = project_pieces(g + 1)
                    qds = None
                elif g == 5:
                    # sg6's V is deferred into att6
                    fillers = p6[0:6]
                    qds = None
                elif g == 6:
                    # v6 + q7 drip here; masked quad waits for v6
                    fillers = p6[6:] + p7[3:6]
                    qds = [0, 1, 2, 3, 6, 4, 5]
                else:
                    # sg7's K/V drip inside att7 itself
                    fillers = p7[0:3] + p7[6:]
                    qds = [0, 1, 2, 3, 7, 4, 5, 6]
                att_run(g, fillers, qds=qds)
                att_finish(g)
                if g == 2:
                    load_x_quarter(3)

    nc.compile()
    return nc


_NC = None
LAST_RESULTS = None


def kernel(x, WQ, WK, WV):
    import os

    import ml_dtypes
    from concourse import bass_utils

    global _NC, LAST_RESULTS
    x = np.asarray(x, dtype=np.float32)
    WQ = np.ascontiguousarray(np.asarray(WQ, dtype=np.float32))
    WK = np.ascontiguousarray(np.asarray(WK, dtype=np.float32))
    WV = np.ascontiguousarray(np.asarray(WV, dtype=np.float32))

    if _NC is None:
        _NC = _build()
    nc = _NC

    f8t = ml_dtypes.float8_e4m3

    def sbuf_layout(w):
        # [E, D] -> [P, ECH*D] with e-chunk ec at columns [ec*D, (ec+1)*D)
        return np.ascontiguousarray(
            w.reshape(ECH, P, D).transpose(1, 0, 2).reshape(P, ECH * D)
        )

    def packed_passes(w):
        # compensated fp8: [W8, rw8] of W*16 (rw8 = unscaled residual)
        w16 = sbuf_layout(w * 16.0)
        w8 = w16.astype(f8t)
        rw8 = (w16 - w8.astype(np.float32)).astype(f8t)
        return np.stack([w8, rw8], axis=1)  # [P, 2, ECH*D]

    wp = np.ascontiguousarray(
        np.stack(
            [packed_passes(WK), packed_passes(WQ), packed_passes(WV)], axis=1
        ).reshape(P, 3 * 2 * ECH * D)
    )

    in_maps = []
    for c in range(8):
        b, h = c >> 1, c & 1
        xb = x[b]  # [S, E]
        # pool permutation: per 512-span u, own 256 queries first
        parts = []
        for u in range(8):
            parts.append(xb[512 * u + 256 * h : 512 * u + 256 * h + 256])
            parts.append(
                xb[512 * u + 256 * (1 - h) : 512 * u + 256 * (1 - h) + 256]
            )
        pool_t = np.concatenate(parts, axis=0).T  # [E, S]
        x8 = pool_t.astype(f8t)
        r8 = (pool_t - x8.astype(np.float32)).astype(f8t)
        x8 = np.ascontiguousarray(x8)
        r8 = np.ascontiguousarray(r8)
        # boot: [K W8 | x8 ec0-1, cols 0:512] per partition
        xpart = (
            x8[: 4 * P, 0:512]
            .reshape(4, P, 512)
            .transpose(1, 0, 2)
            .reshape(P, 2048)
        )
        boot = np.ascontiguousarray(
            np.concatenate([wp[:, 0:1024], xpart], axis=1)
        )
        km = np.zeros((P, 2), dtype=np.float32)
        km[:, 0] = 0.0 if h == 1 else NEG
        km[:, 1] = float(h)
        in_maps.append(
            {"boot": boot, "x8": x8, "r8": r8, "wp": wp, "km": km}
        )

    trace = os.environ.get("KERNEL_TRACE") == "1"
    res = bass_utils.run_bass_kernel_spmd(
        nc, in_maps, core_ids=list(range(8)), trace=trace
    )
    LAST_RESULTS = res

    out = np.empty((B, S, D), dtype=np.float32)
    for c in range(8):
        b, h = c >> 1, c & 1
        raw = res.results[c]["out"]  # [H, D+1]; last column = denominator
        r = raw[:, :D] / raw[:, D:] / 16.0  # V carries 16x from fp8 packing
        for g in range(NQG):
            out[b, 512 * g + 256 * h : 512 * g + 256 * h + 256] = r[
                256 * g : 256 * (g + 1)
            ]
    return out
